# revision 3
# baseline (speedup 1.0000x reference)
"""Single-head causal attention (CustomHead) on 8 Trainium2 NeuronCores.

Reference (per batch b):
    q = x Wq^T ; k = x Wk^T ; v = x Wv^T          (x: [T, C], W*: [H, C])
    S = q k^T * C**-0.5 ; causal mask ; softmax ; out = P v    ([T, H])

Sharding: data-parallel over batch B=32 across 8 cores (4 batches/core).
Each core holds full Wq/Wk/Wv.

Kernel plan per core (T=2048, C=1024, H=128):
  - x loaded with SWDGE cast-DMA (fp32->bf16), then transposed into
    x^T with the DMA XBAR transpose (16 one-instruction [128,1024] ->
    [128,8,128]-slice transposes per batch) -- zero PE involvement.
  - x^T additionally cast to fp8(e4m3) on DVE; q^T/k^T computed with
    fp8 DoubleRow matmuls (K=256 per instruction, Wq/Wk scaled by 16
    to sit in fp8 normal range; the 256x score scale is folded into
    the exp activation scale).  v^T stays bf16 (fp8 v fails accuracy).
  - v^T is transposed back to natural [s, h] layout with one XBAR
    transpose per batch into a strided [128, 16, 144] tile whose col
    128 holds the ones column (rowsum accumulator trick).
  - Scores computed transposed: S^T[s, t] = kT(s-block) vs qT, 512-wide
    PSUM chunks, one exp (ACT) per chunk; no max-subtraction (|S| is
    bounded; exp in fp32 is safe).  P^T rows stored bf16 in
    right-sized tiles ([128, 2048-128*ss]).
  - Causal handling: S^T block-row ss only computes t >= 512*(ss//4);
    the diagonal 128x128 block is masked by an upper-triangular 0/1
    multiply after exp.
  - P.V: accumulated over s-blocks with N=129 matmuls (ones column
    gives the softmax denominator for free); normalize on DVE.
"""

import numpy as np

B, T, C, H = 32, 2048, 1024, 128
NCORES = 8
BL = B // NCORES  # batches per core

_CACHE = {}


def _build():
    import concourse.bass as bass
    import concourse.tile as tile
    from concourse import bacc, mybir
    from concourse.masks import make_upper_triangular

    f32 = mybir.dt.float32
    bf16 = mybir.dt.bfloat16
    f8 = mybir.dt.float8e4
    Exp = mybir.ActivationFunctionType.Exp
    DR = mybir.MatmulPerfMode.DoubleRow
    WS = 16.0  # fp8 weight pre-scale for q/k
    EXPSCALE = float(C) ** -0.5 / (WS * WS)

    nc = bacc.Bacc(
        "TRN2",
        target_bir_lowering=False,
        debug=False,
        enable_asserts=False,
        num_devices=NCORES,
    )
    x_ap = nc.dram_tensor("x", [BL, T, C], f32, kind="ExternalInput").ap()
    wk_ap = nc.dram_tensor("Wk", [H, C], f32, kind="ExternalInput").ap()
    wq_ap = nc.dram_tensor("Wq", [H, C], f32, kind="ExternalInput").ap()
    wv_ap = nc.dram_tensor("Wv", [H, C], f32, kind="ExternalInput").ap()
    out_ap = nc.dram_tensor("out", [BL, T, H], f32, kind="ExternalOutput").ap()

    with tile.TileContext(nc) as tc:
        from contextlib import ExitStack

        with ExitStack() as ctx:
            consts = ctx.enter_context(tc.tile_pool(name="consts", bufs=1))
            wstage = ctx.enter_context(tc.tile_pool(name="wstage", bufs=1))
            xb_p = ctx.enter_context(tc.tile_pool(name="xb", bufs=6))
            xt_p = ctx.enter_context(tc.tile_pool(name="xt", bufs=2))
            x8_p = ctx.enter_context(tc.tile_pool(name="x8", bufs=2))
            qkv_p = ctx.enter_context(tc.tile_pool(name="qkv", bufs=2))
            va_p = ctx.enter_context(tc.tile_pool(name="va", bufs=2))
            pr_p = ctx.enter_context(tc.tile_pool(name="prow", bufs=1))
            osb_p = ctx.enter_context(tc.tile_pool(name="osb", bufs=2))
            rc_p = ctx.enter_context(tc.tile_pool(name="rc", bufs=4))
            mm_ps = ctx.enter_context(tc.tile_pool(name="mm_ps", bufs=2, space="PSUM"))
            srow_ps = ctx.enter_context(
                tc.tile_pool(name="srow_ps", bufs=2, space="PSUM")
            )
            pv_ps = ctx.enter_context(tc.tile_pool(name="pv_ps", bufs=2, space="PSUM"))

            # trimask[s, t] = 1 if s <= t else 0 (valid region of the
            # transposed diagonal block)
            trimask = consts.tile([128, 128], bf16)
            make_upper_triangular(nc, trimask, val=1.0, diag=True)

            # --- weights: load, scale+cast, XBAR-transpose to [c,h] ---
            # wq/wk: bf16 *16 -> transpose -> fp8 [128, 8, 128]
            # wv: bf16 -> transpose -> [128, 8, 128]
            w8 = {}
            for name, wap in (("q", wq_ap), ("k", wk_ap)):
                wnat = wstage.tile([128, C], f32, tag="wnat")
                nc.sync.dma_start(out=wnat, in_=wap)
                wsc = wstage.tile([128, C], bf16, tag="wsc")
                nc.vector.tensor_scalar_mul(wsc, wnat, WS)
                wt3 = wstage.tile([128, 8, 128], bf16, tag="wt3")
                nc.sync.dma_start(out=wt3, in_=wsc, transpose=True)
                wq8 = consts.tile([128, 8, 128], f8, tag=f"w8_{name}", name=f"w8{name}")
                nc.vector.tensor_copy(out=wq8, in_=wt3)
                w8[name] = wq8
            wnat = wstage.tile([128, C], f32, tag="wnat")
            nc.sync.dma_start(out=wnat, in_=wv_ap)
            wvb = wstage.tile([128, C], bf16, tag="wsc")
            nc.vector.tensor_copy(out=wvb, in_=wnat)
            wtv = consts.tile([128, 8, 128], bf16)
            nc.sync.dma_start(out=wtv, in_=wvb, transpose=True)

            for b in range(BL):
                # --- x load: SWDGE cast-DMA fp32 -> bf16, natural layout ---
                xbs = []
                for tt in range(16):
                    xb = xb_p.tile([128, C], bf16, tag="xb", name=f"xb{tt}")
                    nc.gpsimd.dma_start(
                        out=xb, in_=x_ap[b, 128 * tt : 128 * (tt + 1), :]
                    )
                    xbs.append(xb)

                # --- x -> x^T via DMA XBAR (zero PE cost) ---
                xt = xt_p.tile([128, 8, T], bf16)
                for tt in range(16):
                    nc.sync.dma_start(
                        out=xt[:, :, 128 * tt : 128 * (tt + 1)],
                        in_=xbs[tt],
                        transpose=True,
                    )
                # --- fp8 copy of x^T for the q/k projections ---
                x8 = x8_p.tile([128, 8, T], f8)
                for s4 in range(4):
                    nc.vector.tensor_copy(
                        out=x8[:, :, 512 * s4 : 512 * (s4 + 1)],
                        in_=xt[:, :, 512 * s4 : 512 * (s4 + 1)],
                    )

                # --- projections ---
                qT = qkv_p.tile([128, T], bf16, tag="qT")
                kT = qkv_p.tile([128, T], bf16, tag="kT")
                for wt8, dst in ((w8["q"], qT), (w8["k"], kT)):
                    for s4 in range(4):
                        ps = mm_ps.tile([128, 512], f32)
                        for g in range(4):
                            nc.tensor.matmul(
                                ps,
                                wt8[:, 2 * g : 2 * g + 2, :],
                                x8[:, 2 * g : 2 * g + 2, 512 * s4 : 512 * (s4 + 1)],
                                start=(g == 0),
                                stop=(g == 3),
                                perf_mode=DR,
                            )
                        nc.scalar.copy(out=dst[:, 512 * s4 : 512 * (s4 + 1)], in_=ps)
                vT = qkv_p.tile([128, T], bf16, tag="vT")
                for s4 in range(4):
                    ps = mm_ps.tile([128, 512], f32)
                    for cc in range(8):
                        nc.tensor.matmul(
                            ps,
                            wtv[:, cc, :],
                            xt[:, cc, 512 * s4 : 512 * (s4 + 1)],
                            start=(cc == 0),
                            stop=(cc == 7),
                        )
                    nc.scalar.copy(out=vT[:, 512 * s4 : 512 * (s4 + 1)], in_=ps)

                # --- v^T -> natural [s, h] + ones column via XBAR ---
                va = va_p.tile([128, 16, 144], bf16)
                nc.gpsimd.memset(va[:, :, 128:129], 1.0)
                nc.sync.dma_start(out=va[:, :, 0:128], in_=vT, transpose=True)

                # --- scores (transposed), exp, and P.V interleaved ---
                out_sb = osb_p.tile([128, 16 * H], f32)
                prows = []
                for ss in range(16):
                    pb = 128 * ss
                    pr = pr_p.tile(
                        [128, T - pb], bf16, tag=f"pr{ss}", name=f"pr{ss}"
                    )
                    prows.append(pr)
                    for tq in range(ss // 4, 4):
                        c0 = 512 * tq
                        x0 = max(pb, c0)  # first causal-needed column
                        d0 = x0 - c0
                        sh = srow_ps.tile([128, 512], f32)
                        nc.tensor.matmul(
                            sh[:, d0:512],
                            kT[:, pb : pb + 128],
                            qT[:, x0 : c0 + 512],
                            start=True,
                            stop=True,
                        )
                        nc.scalar.activation(
                            out=pr[:, x0 - pb : c0 + 512 - pb],
                            in_=sh[:, d0:512],
                            func=Exp,
                            scale=EXPSCALE,
                        )
                    nc.vector.tensor_mul(pr[:, 0:128], pr[:, 0:128], trimask)
                    pv = pv_ps.tile([128, H + 1], f32)
                    for j in range(ss + 1):
                        nc.tensor.matmul(
                            pv,
                            prows[j][:, pb - 128 * j : pb - 128 * j + 128],
                            va[:, j, 0:129],
                            start=(j == 0),
                            stop=(j == ss),
                        )
                    rc = rc_p.tile([128, 1], f32)
                    nc.vector.reciprocal(rc, pv[:, 128:129])
                    nc.vector.tensor_mul(
                        out_sb[:, H * ss : H * (ss + 1)],
                        pv[:, 0:128],
                        rc.broadcast_to([128, H]),
                    )
                # out_sb[p, (g h)] -> out[b, 128g+p, h]; split DMAs so the
                # final transfer after the last normalize is small
                np_split = 4 if b == BL - 1 else 2
                npc = 2048 // np_split
                for hh in range(np_split):
                    nc.sync.dma_start(
                        out=out_ap[b, npc * hh : npc * (hh + 1), :].rearrange(
                            "(g p) h -> p g h", p=128
                        ),
                        in_=out_sb[
                            :, npc // 128 * H * hh : npc // 128 * H * (hh + 1)
                        ].rearrange("p (g h) -> p g h", h=H),
                    )

    nc.compile()
    return nc


def _get_nc():
    if "nc" not in _CACHE:
        _CACHE["nc"] = _build()
    return _CACHE["nc"]


def kernel(x, Wk, Wq, Wv, _trace=False):
    from concourse.bass_utils import run_bass_kernel_spmd

    x = np.ascontiguousarray(np.asarray(x, dtype=np.float32))
    Wk = np.ascontiguousarray(np.asarray(Wk, dtype=np.float32))
    Wq = np.ascontiguousarray(np.asarray(Wq, dtype=np.float32))
    Wv = np.ascontiguousarray(np.asarray(Wv, dtype=np.float32))
    assert x.shape == (B, T, C)

    nc = _get_nc()
    in_maps = [
        {"x": x[i * BL : (i + 1) * BL], "Wk": Wk, "Wq": Wq, "Wv": Wv}
        for i in range(NCORES)
    ]
    res = run_bass_kernel_spmd(nc, in_maps, list(range(NCORES)), trace=_trace)
    out = np.concatenate([res.results[i]["out"] for i in range(NCORES)], axis=0)
    if _trace:
        _CACHE["last_results"] = res
    return out


# revision 4
# speedup vs baseline: 1.4915x; 1.4915x over previous
"""Single-head causal attention (CustomHead) on 8 Trainium2 NeuronCores.

Reference (per batch b):
    q = x Wq^T ; k = x Wk^T ; v = x Wv^T          (x: [T, C], W*: [H, C])
    S = q k^T * C**-0.5 ; causal mask ; softmax ; out = P v    ([T, H])

Sharding: data-parallel over batch B=32 across 8 cores (4 batches/core).
Each core holds full Wq/Wk/Wv.

Kernel plan per core (T=2048, C=1024, H=128), fp32 accum everywhere:
  - x is loaded with SWDGE cast-DMA (fp32->bf16 in the DMA engine).
  - PE-transpose x into x^T bf16 (every projection contracts over C,
    which must sit on the partition dim); DVE copies PSUM->SBUF.
  - x^T is additionally cast to fp8(e4m3) on GpSimd; q^T/k^T use fp8
    DoubleRow matmuls (K=256/instr, ~1.5x PE throughput).  Wq/Wk are
    pre-scaled by 16 so their 0.02-std weights sit in fp8 normal
    range; the resulting 256x score scale is folded into the exp
    activation scale.  v^T stays bf16 (fp8 anywhere in the v path
    fails the accuracy budget).
  - v^T is PE-transposed back to natural [s, h] blocks.
  - Scores computed transposed: S^T[s, t] = kT(s-block) vs qT, 512-wide
    PSUM chunks, one exp (ACT) per chunk.  No max-subtraction (scores
    are bounded; exp is safe in fp32); the row-sum comes free from a
    ones-column appended to v (P^T @ [v | 1] accumulates numerator and
    denominator together).
  - Causal handling: S^T block-row ss only computes t >= 512*(ss//4);
    the diagonal 128x128 block is masked by an upper-triangular 0/1
    multiply after exp; everything below is never read.  P^T rows are
    stored in right-sized tiles ([128, 2048-128*ss]).
  - Output rows are normalized into one SBUF tile per batch and written
    with split DMAs.
"""

import numpy as np

B, T, C, H = 32, 2048, 1024, 128
NCORES = 8
BL = B // NCORES  # batches per core

_CACHE = {}


def _build():
    import concourse.bass as bass
    import concourse.tile as tile
    from concourse import bacc, mybir
    from concourse.masks import make_identity, make_upper_triangular

    f32 = mybir.dt.float32
    bf16 = mybir.dt.bfloat16
    f8 = mybir.dt.float8e4
    Exp = mybir.ActivationFunctionType.Exp
    DR = mybir.MatmulPerfMode.DoubleRow
    WS = 16.0  # fp8 pre-scale for Wq/Wk
    EXPSCALE = float(C) ** -0.5 / (WS * WS)

    nc = bacc.Bacc(
        "TRN2",
        target_bir_lowering=False,
        debug=False,
        enable_asserts=False,
        num_devices=NCORES,
    )
    x_ap = nc.dram_tensor("x", [BL, T, C], f32, kind="ExternalInput").ap()
    wk_ap = nc.dram_tensor("Wk", [H, C], f32, kind="ExternalInput").ap()
    wq_ap = nc.dram_tensor("Wq", [H, C], f32, kind="ExternalInput").ap()
    wv_ap = nc.dram_tensor("Wv", [H, C], f32, kind="ExternalInput").ap()
    out_ap = nc.dram_tensor("out", [BL, T, H], f32, kind="ExternalOutput").ap()

    with tile.TileContext(nc) as tc:
        from contextlib import ExitStack

        with ExitStack() as ctx:
            consts = ctx.enter_context(tc.tile_pool(name="consts", bufs=1))
            wstage = ctx.enter_context(tc.tile_pool(name="wstage", bufs=1))
            xbf_p = ctx.enter_context(tc.tile_pool(name="xbf", bufs=16))
            xt_p = ctx.enter_context(tc.tile_pool(name="xt", bufs=9))
            x8_p = ctx.enter_context(tc.tile_pool(name="x8", bufs=2))
            qk_p = ctx.enter_context(tc.tile_pool(name="qk", bufs=2))
            va_p = ctx.enter_context(tc.tile_pool(name="va", bufs=20))
            pr_p = ctx.enter_context(tc.tile_pool(name="prow", bufs=1))
            osb_p = ctx.enter_context(tc.tile_pool(name="osb", bufs=2))
            rc_p = ctx.enter_context(tc.tile_pool(name="rc", bufs=4))
            trans_ps = ctx.enter_context(
                tc.tile_pool(name="trans_ps", bufs=2, space="PSUM")
            )
            mm_ps = ctx.enter_context(tc.tile_pool(name="mm_ps", bufs=2, space="PSUM"))
            srow_ps = ctx.enter_context(
                tc.tile_pool(name="srow_ps", bufs=2, space="PSUM")
            )
            pv_ps = ctx.enter_context(tc.tile_pool(name="pv_ps", bufs=2, space="PSUM"))

            ident = consts.tile([128, 128], bf16)
            make_identity(nc, ident)

            # trimask[s, t] = 1 if s <= t else 0 (valid region of the
            # transposed diagonal block)
            trimask = consts.tile([128, 128], bf16)
            make_upper_triangular(nc, trimask, val=1.0, diag=True)

            # --- weights ---
            # wq/wk: *16, cast bf16, PE-transpose to [c,h], cast fp8
            # wv: cast bf16, PE-transpose to [c,h] chunks
            W8 = {}
            for name, wap in (("q", wq_ap), ("k", wk_ap)):
                wnat = wstage.tile([128, C], f32, tag="wnat")
                nc.sync.dma_start(out=wnat, in_=wap)
                wbf = wstage.tile([128, C], bf16, tag="wbf")
                nc.vector.tensor_scalar_mul(wbf, wnat, WS)
                wt = wstage.tile([128, C], bf16, tag="wt")
                for g in range(2):
                    ps = trans_ps.tile([128, 512], bf16)
                    for m in range(4):
                        cc = 4 * g + m
                        nc.tensor.transpose(
                            ps[:, 128 * m : 128 * (m + 1)],
                            wbf[:, 128 * cc : 128 * (cc + 1)],
                            ident,
                        )
                    nc.vector.tensor_copy(out=wt[:, 512 * g : 512 * (g + 1)], in_=ps)
                w8 = consts.tile([128, 8, 128], f8, tag=f"w8{name}", name=f"w8{name}")
                nc.vector.tensor_copy(out=w8, in_=wt.rearrange("p (cc h) -> p cc h", h=128))
                W8[name] = w8
            wnat = wstage.tile([128, C], f32, tag="wnat")
            nc.sync.dma_start(out=wnat, in_=wv_ap)
            wbf = wstage.tile([128, C], bf16, tag="wbf")
            nc.vector.tensor_copy(out=wbf, in_=wnat)
            wtv = consts.tile([128, C], bf16)
            for g in range(2):
                ps = trans_ps.tile([128, 512], bf16)
                for m in range(4):
                    cc = 4 * g + m
                    nc.tensor.transpose(
                        ps[:, 128 * m : 128 * (m + 1)],
                        wbf[:, 128 * cc : 128 * (cc + 1)],
                        ident,
                    )
                nc.vector.tensor_copy(out=wtv[:, 512 * g : 512 * (g + 1)], in_=ps)

            for b in range(BL):
                # --- x load: SWDGE cast-DMA fp32 -> bf16, natural layout ---
                xbfs = []
                for tt in range(16):
                    xb = xbf_p.tile([128, C], bf16, tag="xb", name=f"xb{tt}")
                    nc.gpsimd.dma_start(
                        out=xb, in_=x_ap[b, 128 * tt : 128 * (tt + 1), :]
                    )
                    xbfs.append(xb)

                # --- x -> x^T (bf16) via PE transpose ---
                xts = [
                    xt_p.tile([128, T], bf16, name=f"xt{cc}", tag="xt")
                    for cc in range(8)
                ]
                for tt8 in range(2):
                    for cc in range(8):
                        ps = trans_ps.tile([128, 1024], bf16)
                        for m in range(8):
                            nc.tensor.transpose(
                                ps[:, 128 * m : 128 * (m + 1)],
                                xbfs[8 * tt8 + m][:, 128 * cc : 128 * (cc + 1)],
                                ident,
                            )
                        nc.vector.tensor_copy(
                            out=xts[cc][:, 1024 * tt8 : 1024 * (tt8 + 1)], in_=ps
                        )

                # --- fp8 copy of x^T (GpSimd) for the q/k projections ---
                x8 = x8_p.tile([128, 8, T], f8)
                for cc in range(8):
                    for tt8 in range(2):
                        nc.gpsimd.tensor_copy(
                            out=x8[:, cc, 1024 * tt8 : 1024 * (tt8 + 1)],
                            in_=xts[cc][:, 1024 * tt8 : 1024 * (tt8 + 1)],
                        )

                # --- q/k projections: fp8 DoubleRow (K=256 per matmul) ---
                qT = qk_p.tile([128, T], bf16, tag="qT")
                kT = qk_p.tile([128, T], bf16, tag="kT")
                for w8, dst in ((W8["q"], qT), (W8["k"], kT)):
                    for s4 in range(4):
                        ps = mm_ps.tile([128, 512], f32)
                        for g in range(4):
                            nc.tensor.matmul(
                                ps,
                                w8[:, 2 * g : 2 * g + 2, :],
                                x8[:, 2 * g : 2 * g + 2, 512 * s4 : 512 * (s4 + 1)],
                                start=(g == 0),
                                stop=(g == 3),
                                perf_mode=DR,
                            )
                        nc.scalar.copy(out=dst[:, 512 * s4 : 512 * (s4 + 1)], in_=ps)
                # v^T = Wv @ x^T (bf16), then PE-transpose back to natural
                vT = qk_p.tile([128, T], bf16, tag="vT")
                for s4 in range(4):
                    ps = mm_ps.tile([128, 512], f32)
                    for cc in range(8):
                        nc.tensor.matmul(
                            ps,
                            wtv[:, 128 * cc : 128 * (cc + 1)],
                            xts[cc][:, 512 * s4 : 512 * (s4 + 1)],
                            start=(cc == 0),
                            stop=(cc == 7),
                        )
                    nc.scalar.copy(out=vT[:, 512 * s4 : 512 * (s4 + 1)], in_=ps)
                vas = []
                for ss in range(16):
                    psv = trans_ps.tile([128, 512], bf16, tag="ps")
                    nc.tensor.transpose(
                        psv[:, 0:128], vT[:, 128 * ss : 128 * (ss + 1)], ident
                    )
                    va = va_p.tile([128, H + 1], bf16)
                    nc.vector.tensor_copy(out=va[:, 0:128], in_=psv[:, 0:128])
                    nc.gpsimd.memset(va[:, 128:129], 1.0)
                    vas.append(va)

                # --- scores (transposed), exp, and P.V interleaved ---
                out_sb = osb_p.tile([128, 16 * H], f32)
                prows = []
                for ss in range(16):
                    pb = 128 * ss
                    pr = pr_p.tile([128, T - pb], bf16, tag=f"pr{ss}", name=f"pr{ss}")
                    prows.append(pr)
                    for tq in range(ss // 4, 4):
                        c0 = 512 * tq
                        x0 = max(pb, c0)  # first causal-needed column
                        d0 = x0 - c0
                        sh = srow_ps.tile([128, 512], f32)
                        nc.tensor.matmul(
                            sh[:, d0:512],
                            kT[:, pb : pb + 128],
                            qT[:, x0 : c0 + 512],
                            start=True,
                            stop=True,
                        )
                        nc.scalar.activation(
                            out=pr[:, x0 - pb : c0 + 512 - pb],
                            in_=sh[:, d0:512],
                            func=Exp,
                            scale=EXPSCALE,
                        )
                    nc.vector.tensor_mul(pr[:, 0:128], pr[:, 0:128], trimask)
                    pv = pv_ps.tile([128, H + 1], f32)
                    for j in range(ss + 1):
                        nc.tensor.matmul(
                            pv,
                            prows[j][:, pb - 128 * j : pb - 128 * j + 128],
                            vas[j],
                            start=(j == 0),
                            stop=(j == ss),
                        )
                    rc = rc_p.tile([128, 1], f32)
                    nc.vector.reciprocal(rc, pv[:, 128:129])
                    nc.vector.tensor_mul(
                        out_sb[:, H * ss : H * (ss + 1)],
                        pv[:, 0:128],
                        rc.broadcast_to([128, H]),
                    )
                # out_sb[p, (g h)] -> out[b, 128g+p, h]; split DMAs so the
                # final transfer after the last normalize is small
                np_split = 4 if b == BL - 1 else 2
                npc = 2048 // np_split
                for hh in range(np_split):
                    nc.sync.dma_start(
                        out=out_ap[b, npc * hh : npc * (hh + 1), :].rearrange(
                            "(g p) h -> p g h", p=128
                        ),
                        in_=out_sb[
                            :, npc // 128 * H * hh : npc // 128 * H * (hh + 1)
                        ].rearrange("p (g h) -> p g h", h=H),
                    )

    nc.compile()
    return nc


def _get_nc():
    if "nc" not in _CACHE:
        _CACHE["nc"] = _build()
    return _CACHE["nc"]


def kernel(x, Wk, Wq, Wv, _trace=False):
    from concourse.bass_utils import run_bass_kernel_spmd

    x = np.ascontiguousarray(np.asarray(x, dtype=np.float32))
    Wk = np.ascontiguousarray(np.asarray(Wk, dtype=np.float32))
    Wq = np.ascontiguousarray(np.asarray(Wq, dtype=np.float32))
    Wv = np.ascontiguousarray(np.asarray(Wv, dtype=np.float32))
    assert x.shape == (B, T, C)

    nc = _get_nc()
    in_maps = [
        {"x": x[i * BL : (i + 1) * BL], "Wk": Wk, "Wq": Wq, "Wv": Wv}
        for i in range(NCORES)
    ]
    res = run_bass_kernel_spmd(nc, in_maps, list(range(NCORES)), trace=_trace)
    out = np.concatenate([res.results[i]["out"] for i in range(NCORES)], axis=0)
    if _trace:
        _CACHE["last_results"] = res
    return out


# revision 9
# speedup vs baseline: 2.2268x; 1.4930x over previous
"""Single-head causal attention (CustomHead) on 8 Trainium2 NeuronCores.

Reference (per batch b):
    q = x Wq^T ; k = x Wk^T ; v = x Wv^T          (x: [T, C], W*: [H, C])
    S = q k^T * C**-0.5 ; causal mask ; softmax ; out = P v    ([T, H])

Sharding: data-parallel over batch B=32 across 8 cores (4 batches/core).
Each core holds full Wq/Wk/Wv.

Kernel plan per core (T=2048, C=1024, H=128), fp32 accum everywhere:
  - x is loaded with SWDGE cast-DMA (fp32->bf16 in the DMA engine).
  - PE-transpose x into x^T bf16 (every projection contracts over C,
    which must sit on the partition dim); DVE copies PSUM->SBUF.
  - x^T is additionally cast to fp8(e4m3) on GpSimd; q^T/k^T use fp8
    DoubleRow matmuls (K=256/instr, ~1.5x PE throughput).  Wq/Wk are
    pre-scaled by 16 so their 0.02-std weights sit in fp8 normal
    range; the resulting 256x score scale is folded into the exp
    activation scale.  v^T stays bf16 (fp8 anywhere in the v path
    fails the accuracy budget).
  - v^T is PE-transposed back to natural [s, h] blocks.
  - Scores computed transposed: S^T[s, t] = kT(s-block) vs qT, 512-wide
    PSUM chunks, one exp (ACT) per chunk.  No max-subtraction (scores
    are bounded; exp is safe in fp32); the row-sum comes free from a
    ones-column appended to v (P^T @ [v | 1] accumulates numerator and
    denominator together).
  - Causal handling: S^T block-row ss only computes t >= 512*(ss//4);
    the diagonal 128x128 block is masked by an upper-triangular 0/1
    multiply after exp; everything below is never read.  P^T rows are
    stored in right-sized tiles ([128, 2048-128*ss]).
  - Output rows are normalized into one SBUF tile per batch and written
    with split DMAs.
"""

import numpy as np

B, T, C, H = 32, 2048, 1024, 128
NCORES = 8
BL = B // NCORES  # batches per core

_CACHE = {}


def _build():
    import concourse.bass as bass
    import concourse.tile as tile
    from concourse import bacc, mybir
    from concourse.masks import make_identity, make_upper_triangular

    f32 = mybir.dt.float32
    bf16 = mybir.dt.bfloat16
    f8 = mybir.dt.float8e4
    Exp = mybir.ActivationFunctionType.Exp
    DR = mybir.MatmulPerfMode.DoubleRow
    WS = 16.0  # fp8 pre-scale for Wq/Wk
    EXPSCALE = float(C) ** -0.5 / (WS * WS)

    nc = bacc.Bacc(
        "TRN2",
        target_bir_lowering=False,
        debug=False,
        enable_asserts=False,
        num_devices=NCORES,
    )
    x_ap = nc.dram_tensor("x", [BL, T, C], f32, kind="ExternalInput").ap()
    wk_ap = nc.dram_tensor("Wk", [H, C], f32, kind="ExternalInput").ap()
    wq_ap = nc.dram_tensor("Wq", [H, C], f32, kind="ExternalInput").ap()
    wv_ap = nc.dram_tensor("Wv", [H, C], f32, kind="ExternalInput").ap()
    out_ap = nc.dram_tensor("out", [BL, T, H], f32, kind="ExternalOutput").ap()

    with tile.TileContext(nc) as tc:
        from contextlib import ExitStack

        with ExitStack() as ctx:
            consts = ctx.enter_context(tc.tile_pool(name="consts", bufs=1))
            wstage = ctx.enter_context(tc.tile_pool(name="wstage", bufs=1))
            xbf_p = ctx.enter_context(tc.tile_pool(name="xbf", bufs=16))
            xt_p = ctx.enter_context(tc.tile_pool(name="xt", bufs=9))
            x8_p = ctx.enter_context(tc.tile_pool(name="x8", bufs=2))
            qk_p = ctx.enter_context(tc.tile_pool(name="qk", bufs=2))
            va_p = ctx.enter_context(tc.tile_pool(name="va", bufs=2))
            pr_p = ctx.enter_context(tc.tile_pool(name="prow", bufs=1))
            osb_p = ctx.enter_context(tc.tile_pool(name="osb", bufs=2))
            rc_p = ctx.enter_context(tc.tile_pool(name="rc", bufs=4))
            trans_ps = ctx.enter_context(
                tc.tile_pool(name="trans_ps", bufs=2, space="PSUM")
            )
            mm_ps = ctx.enter_context(tc.tile_pool(name="mm_ps", bufs=2, space="PSUM"))
            srow_ps = ctx.enter_context(
                tc.tile_pool(name="srow_ps", bufs=2, space="PSUM")
            )
            pv_ps = ctx.enter_context(tc.tile_pool(name="pv_ps", bufs=2, space="PSUM"))

            ident = consts.tile([128, 128], bf16)
            make_identity(nc, ident)

            # trimask[s, t] = 1 if s <= t else 0 (valid region of the
            # transposed diagonal block)
            trimask = consts.tile([128, 128], bf16)
            make_upper_triangular(nc, trimask, val=1.0, diag=True)

            # --- weights: load, scale+cast, XBAR-transpose to [c%128, cc, h] ---
            # wq/wk additionally cast to fp8 (pre-scaled by 16)
            W8 = {}
            for name, wap in (("q", wq_ap), ("k", wk_ap)):
                wnat = wstage.tile([128, C], f32, tag="wnat")
                nc.sync.dma_start(out=wnat, in_=wap)
                wbf = wstage.tile([128, C], bf16, tag="wbf")
                nc.vector.tensor_scalar_mul(wbf, wnat, WS)
                wt3 = wstage.tile([128, 8, 128], bf16, tag="wt3")
                nc.sync.dma_start(out=wt3, in_=wbf, transpose=True)
                w8 = consts.tile([128, 8, 128], f8, tag=f"w8{name}", name=f"w8{name}")
                nc.vector.tensor_copy(out=w8, in_=wt3)
                W8[name] = w8
            wnat = wstage.tile([128, C], f32, tag="wnat")
            nc.sync.dma_start(out=wnat, in_=wv_ap)
            wbf = wstage.tile([128, C], bf16, tag="wbf")
            nc.vector.tensor_copy(out=wbf, in_=wnat)
            wtv = consts.tile([128, 8, 128], bf16)
            nc.sync.dma_start(out=wtv, in_=wbf, transpose=True)

            for b in range(BL):
                # --- x load: SWDGE cast-DMA fp32 -> bf16, natural layout ---
                xbfs = []
                for tt in range(16):
                    xb = xbf_p.tile([128, C], bf16, tag="xb", name=f"xb{tt}")
                    nc.gpsimd.dma_start(
                        out=xb, in_=x_ap[b, 128 * tt : 128 * (tt + 1), :]
                    )
                    xbfs.append(xb)

                # --- x -> x^T (bf16) via PE transpose ---
                xts = [
                    xt_p.tile([128, T], bf16, name=f"xt{cc}", tag="xt")
                    for cc in range(8)
                ]
                for tt8 in range(2):
                    for cc in range(8):
                        ps = trans_ps.tile([128, 1024], bf16)
                        for m in range(8):
                            nc.tensor.transpose(
                                ps[:, 128 * m : 128 * (m + 1)],
                                xbfs[8 * tt8 + m][:, 128 * cc : 128 * (cc + 1)],
                                ident,
                            )
                        nc.vector.tensor_copy(
                            out=xts[cc][:, 1024 * tt8 : 1024 * (tt8 + 1)], in_=ps
                        )

                # --- fp8 copy of x^T (DVE) for the q/k projections ---
                x8 = x8_p.tile([128, 8, T], f8)
                for cc in range(8):
                    for tt8 in range(2):
                        nc.vector.tensor_copy(
                            out=x8[:, cc, 1024 * tt8 : 1024 * (tt8 + 1)],
                            in_=xts[cc][:, 1024 * tt8 : 1024 * (tt8 + 1)],
                        )

                # --- q/k projections: fp8 DoubleRow (K=256 per matmul) ---
                qT = qk_p.tile([128, T], bf16, tag="qT")
                kT = qk_p.tile([128, T], bf16, tag="kT")
                for w8, dst in ((W8["q"], qT), (W8["k"], kT)):
                    for s4 in range(4):
                        ps = mm_ps.tile([128, 512], f32)
                        for g in range(4):
                            nc.tensor.matmul(
                                ps,
                                w8[:, 2 * g : 2 * g + 2, :],
                                x8[:, 2 * g : 2 * g + 2, 512 * s4 : 512 * (s4 + 1)],
                                start=(g == 0),
                                stop=(g == 3),
                                perf_mode=DR,
                            )
                        nc.scalar.copy(out=dst[:, 512 * s4 : 512 * (s4 + 1)], in_=ps)
                # v^T = Wv @ x^T (bf16), then PE-transpose back to natural
                vT = qk_p.tile([128, T], bf16, tag="vT")
                for s4 in range(4):
                    ps = mm_ps.tile([128, 512], f32)
                    for cc in range(8):
                        nc.tensor.matmul(
                            ps,
                            wtv[:, cc, :],
                            xts[cc][:, 512 * s4 : 512 * (s4 + 1)],
                            start=(cc == 0),
                            stop=(cc == 7),
                        )
                    nc.scalar.copy(out=vT[:, 512 * s4 : 512 * (s4 + 1)], in_=ps)
                # v^T -> natural [s, h] blocks + ones column, one XBAR instr
                va = va_p.tile([128, 16, 144], bf16)
                nc.gpsimd.memset(va[:, :, 128:129], 1.0)
                nc.sync.dma_start(out=va[:, :, 0:128], in_=vT, transpose=True)

                # --- scores (transposed), exp, and P.V interleaved ---
                out_sb = osb_p.tile([128, 16 * H], f32)
                prows = []
                for ss in range(16):
                    pb = 128 * ss
                    pr = pr_p.tile([128, T - pb], bf16, tag=f"pr{ss}", name=f"pr{ss}")
                    prows.append(pr)
                    for tq in range(ss // 4, 4):
                        c0 = 512 * tq
                        x0 = max(pb, c0)  # first causal-needed column
                        d0 = x0 - c0
                        sh = srow_ps.tile([128, 512], f32)
                        nc.tensor.matmul(
                            sh[:, d0:512],
                            kT[:, pb : pb + 128],
                            qT[:, x0 : c0 + 512],
                            start=True,
                            stop=True,
                        )
                        nc.scalar.activation(
                            out=pr[:, x0 - pb : c0 + 512 - pb],
                            in_=sh[:, d0:512],
                            func=Exp,
                            scale=EXPSCALE,
                        )
                    nc.vector.tensor_mul(pr[:, 0:128], pr[:, 0:128], trimask)
                    pv = pv_ps.tile([128, H + 1], f32)
                    for j in range(ss + 1):
                        nc.tensor.matmul(
                            pv,
                            prows[j][:, pb - 128 * j : pb - 128 * j + 128],
                            va[:, j, 0 : H + 1],
                            start=(j == 0),
                            stop=(j == ss),
                        )
                    rc = rc_p.tile([128, 1], f32)
                    nc.vector.reciprocal(rc, pv[:, 128:129])
                    nc.vector.tensor_mul(
                        out_sb[:, H * ss : H * (ss + 1)],
                        pv[:, 0:128],
                        rc.broadcast_to([128, H]),
                    )
                # out_sb[p, (g h)] -> out[b, 128g+p, h]; split DMAs so the
                # final transfer after the last normalize is small
                np_split = 4 if b == BL - 1 else 2
                npc = 2048 // np_split
                for hh in range(np_split):
                    nc.sync.dma_start(
                        out=out_ap[b, npc * hh : npc * (hh + 1), :].rearrange(
                            "(g p) h -> p g h", p=128
                        ),
                        in_=out_sb[
                            :, npc // 128 * H * hh : npc // 128 * H * (hh + 1)
                        ].rearrange("p (g h) -> p g h", h=H),
                    )

    nc.compile()
    return nc


def _get_nc():
    if "nc" not in _CACHE:
        _CACHE["nc"] = _build()
    return _CACHE["nc"]


def kernel(x, Wk, Wq, Wv, _trace=False):
    from concourse.bass_utils import run_bass_kernel_spmd

    x = np.ascontiguousarray(np.asarray(x, dtype=np.float32))
    Wk = np.ascontiguousarray(np.asarray(Wk, dtype=np.float32))
    Wq = np.ascontiguousarray(np.asarray(Wq, dtype=np.float32))
    Wv = np.ascontiguousarray(np.asarray(Wv, dtype=np.float32))
    assert x.shape == (B, T, C)

    nc = _get_nc()
    in_maps = [
        {"x": x[i * BL : (i + 1) * BL], "Wk": Wk, "Wq": Wq, "Wv": Wv}
        for i in range(NCORES)
    ]
    res = run_bass_kernel_spmd(nc, in_maps, list(range(NCORES)), trace=_trace)
    out = np.concatenate([res.results[i]["out"] for i in range(NCORES)], axis=0)
    if _trace:
        _CACHE["last_results"] = res
    return out
